# revision 38
# baseline (speedup 1.0000x reference)
"""LowRankSparse2to4Linear Trainium2 kernel.

out = (x16 @ A16) -> fp16 -> (@ B16^T) + bias, where A16/B16 are the 2:4
soft-thresholded (along rank), scaled, fp16-cast low-rank factors.

Strategy (8 NeuronCores, data-parallel over tokens, NO collectives):
  - tokens (8192) sharded 1024/core; every core receives the FULL weights
    and redundantly preprocesses them on-chip.
  - The tensor engine runs ONLY the two GEMM streams (fp16, 512-row
    matmuls, 8-bank PSUM accumulation sweeps). All transposes (x and
    weight_B) run on the DMA XBAR via dma_start_transpose (fp16,
    SBUF->SBUF), so the PE never transposes and no PSUM->SBUF transpose
    copies exist.
  - 2:4 soft-threshold per 128x1024 chunk: one strided ACT activation
    deinterleaves rank lanes (fused scale + f32->f16 cast) into
    "deint" layout w16d[:, 256*i + q] = w*scale[:, 4*q + i]; then the
    threshold network runs on contiguous fp16: P = custom MINABS of the
    two 512-halves (DVE), Q = standard abs_max tensor_tensor (DVE, 2x
    mode), E/F/t min/max on GPSIMD (otherwise idle), and a fused custom
    SOFT_SHRINK (DVE) with t broadcast via a stride-0 outer AP. The
    rank permutation is consistent between A and B^T so it cancels in
    GEMM2's contraction.
  - x f32->f16 casts run on GPSIMD; GEMM1 consumes DMA-transposed xT
    tiles; GEMM2 consumes DMA-transposed wbT quarter tiles built JIT.
  - PE warmup matmuls (on zeros) run from t=0 so the PE p-state is
    fully ramped (2.4 GHz) when GEMM1 starts.
"""

import os
import sys
import numpy as np

sys.path.insert(0, "/opt/trn_rl_repo")

N_CORES = 8
IN_F, OUT_F, RANK = 4096, 4096, 1024
T_FULL = 8192             # 4 * 2048 tokens
TPC = T_FULL // N_CORES   # 1024 tokens per core

_BUILD_CACHE = {}

_DVE_OPS = {}


def _register_custom_dve_ops():
    """Register fused DVE ops (runtime extension of concourse.dve_ops).

    MINABS:      out = min(|in0|, |in1|)
    SOFT_SHRINK: out = in0 - clamp(in0, -in1, in1)   (in1 >= 0)
    """
    if _DVE_OPS:
        return _DVE_OPS
    import numpy as _np
    from concourse import dve_ops
    from concourse.dve_spec import (Spec, Src0, Src1, Zero, minn, maxx,
                                    select, lower, _has_src1)
    from concourse.dve_uop import DveOpSpec

    def make_op(name, body, ref):
        existing = {op.name: op for op in dve_ops.OPS}
        if name in existing:
            return existing[name]
        spec = Spec(body=body, reference=ref)
        row = dve_ops._CUSTOM_DVE_ROW_BASE + len(dve_ops.OPS)
        shas = {}
        for ver in ("v3", "v4"):
            try:
                tmp = DveOpSpec(name=name, opcode=row, uops=lower(spec, ver=ver),
                                rd1_en=_has_src1(spec))
                shas[ver] = tmp.sha(ver)
            except Exception:
                pass
        op = dve_ops.DveOp(name, spec, subdim=False, uops_sha=shas)
        dve_ops.OPS.append(op)
        dve_ops.CUSTOM_DVE_SPECS[name] = spec
        dve_ops._SUB_OPCODE_FOR_NAME[name] = row
        return op

    _DVE_OPS["minabs"] = make_op(
        "MINABS_ANT", minn(maxx(Src0, Zero - Src0), maxx(Src1, Zero - Src1)),
        lambda in0, in1, s0, s1, imm2: _np.minimum(_np.abs(in0), _np.abs(in1)))
    _DVE_OPS["maxabs"] = make_op(
        "MAXABS_ANT", maxx(maxx(Src0, Zero - Src0), maxx(Src1, Zero - Src1)),
        lambda in0, in1, s0, s1, imm2: _np.maximum(_np.abs(in0), _np.abs(in1)))
    _DVE_OPS["shrink"] = make_op(
        "SOFT_SHRINK_ANT",
        select(Src0 < Zero, minn(Src0 + Src1, Zero), maxx(Src0 - Src1, Zero)),
        lambda in0, in1, s0, s1, imm2: _np.where(
            in0 < 0, _np.minimum(in0 + in1, 0), _np.maximum(in0 - in1, 0)))
    return _DVE_OPS


N_WARMUP = 24  # PE p-state warmup matmuls (512 rows each)


def _build(scale_a: float, scale_b: float, bias_zero: bool):
    import concourse.bacc as bacc
    import concourse.tile as tile
    from concourse import mybir

    ops = _register_custom_dve_ops()

    f32 = mybir.dt.float32
    f16 = mybir.dt.float16
    Alu = mybir.AluOpType
    AF = mybir.ActivationFunctionType

    nc = bacc.Bacc("TRN2", target_bir_lowering=False, debug=False,
                   num_devices=N_CORES)

    x_sh = nc.dram_tensor("x_sh", [TPC, IN_F], f32, kind="ExternalInput")
    wa_d = nc.dram_tensor("wa_d", [IN_F, RANK], f32, kind="ExternalInput")
    wb_d = nc.dram_tensor("wb_d", [OUT_F, RANK], f32, kind="ExternalInput")
    bias_d = nc.dram_tensor("bias_d", [1, OUT_F], f32, kind="ExternalInput")
    out_d = nc.dram_tensor("out_d", [TPC, OUT_F], f32, kind="ExternalOutput")

    K_IN = IN_F // 128    # 32 contraction chunks for GEMM1
    K_RK = RANK // 128    # 8 contraction chunks for GEMM2

    with tile.TileContext(nc) as tc:
        with (
            tc.tile_pool(name="singles", bufs=1) as singles,
            tc.tile_pool(name="wst", bufs=3) as p_wst,      # w f32 staging
            tc.tile_pool(name="xst", bufs=3) as p_xst,      # x f32 staging
            tc.tile_pool(name="x16", bufs=4) as p_x16,
            # xT tiles and wbT quarter tiles share one pool: the wbT
            # quarters reuse the SBUF slots of xT tiles the moment the
            # corresponding GEMM1 sweep stops reading them.
            tc.tile_pool(name="big", bufs=4) as p_big,      # [128,8192] f16
            tc.tile_pool(name="wa", bufs=32) as p_wa,       # resident A
            tc.tile_pool(name="wb", bufs=6) as p_wb,        # JIT B chunks
            tc.tile_pool(name="pq", bufs=4) as p_pq,
            tc.tile_pool(name="eft", bufs=4) as p_eft,
            tc.tile_pool(name="xp", bufs=16) as p_xp,
            tc.tile_pool(name="oev", bufs=2) as p_out,
            tc.tile_pool(name="ps", bufs=8, space="PSUM") as p_ps,
        ):
            # ---- PE warmup: keep the PE busy (p-state ramp) from t=0 ----
            zeros = singles.tile([128, 512], f16)
            nc.vector.memset(zeros[:], 0.0)
            warm = p_ps.tile([128, 512], f32, tag="ps", name="warm")
            for i in range(N_WARMUP):
                nc.tensor.matmul(warm[:], zeros[:, 0:128], zeros[:],
                                 start=(i == 0), stop=(i == N_WARMUP - 1))

            # ---- bias broadcast (log-doubling), only if bias nonzero ----
            if not bias_zero:
                bias_bc = singles.tile([128, OUT_F], f32)
                nc.sync.dma_start(bias_bc[0:1, :], bias_d[:])
                k = 1
                while k < 128:
                    nc.sync.dma_start(bias_bc[k:2 * k, :], bias_bc[0:k, :])
                    k *= 2

            def w_load(src_dram, row0, name):
                st = p_wst.tile([128, RANK], f32, tag="wst",
                                name=f"wst_{name}")
                nc.sync.dma_start(st[:], src_dram[row0:row0 + 128, :])
                return st

            def soft24_chunk(st, scale, dst_pool, name):
                """2:4 soft-threshold one (128, RANK) f32 row chunk into an
                fp16 tile in the deinterleaved rank layout:
                out[:, 256*i + q] = soft(scale*w)[:, 4*q + i]."""
                # fused deinterleave + scale + f32->f16 cast on ACT
                wd = dst_pool.tile([128, RANK], f16, tag="wsp",
                                   name=f"wsp_{name}")
                nc.scalar.activation(
                    wd[:].rearrange("p (f q) -> p f q", f=4),
                    st[:].rearrange("p (q f) -> p f q", f=4),
                    AF.Copy, scale=float(scale))
                # threshold network, all contiguous fp16
                P = p_pq.tile([128, 512], f16, tag="pq", name=f"P_{name}")
                Q = p_pq.tile([128, 512], f16, tag="pq", name=f"Q_{name}")
                nc.vector._custom_dve(ops["minabs"], out=P[:],
                                      in0=wd[:, 0:512], in1=wd[:, 512:1024])
                nc.vector._custom_dve(ops["maxabs"], out=Q[:],
                                      in0=wd[:, 0:512], in1=wd[:, 512:1024])
                E = p_eft.tile([128, 256], f16, tag="eft", name=f"E_{name}")
                F = p_eft.tile([128, 256], f16, tag="eft", name=f"F_{name}")
                t = p_eft.tile([128, 256], f16, tag="eft", name=f"t_{name}")
                nc.vector.tensor_tensor(out=E[:], in0=P[:, 0:256],
                                        in1=P[:, 256:512], op=Alu.max)
                nc.vector.tensor_tensor(out=F[:], in0=Q[:, 0:256],
                                        in1=Q[:, 256:512], op=Alu.min)
                nc.vector.tensor_tensor(out=t[:], in0=E[:], in1=F[:],
                                        op=Alu.min)
                # sp = w - clamp(w, -t, t), fused, all 4 lanes in one op
                nc.vector._custom_dve(
                    ops["shrink"],
                    out=wd[:].rearrange("p (f q) -> p f q", f=4),
                    in0=wd[:].rearrange("p (f q) -> p f q", f=4),
                    in1=t[:, None, :].to_broadcast([128, 4, 256]))
                return wd

            # ---------------- producer phase: A + x + B ----------------
            # x pipeline tiles: (tc tok-chunk 0..7, pair in-col-pair 0..1)
            # xT big tiles: one per (th, pair): [128, 16*512] f16
            xTt = {}

            def x_tile(tc_, hb):
                """load + cast + dma-transpose one (128 tok, 1024 in) tile"""
                pr, h = hb // 2, hb % 2
                xs = p_xst.tile([128, 1024], f32, tag="xst",
                                name=f"xs_{tc_}_{hb}")
                nc.sync.dma_start(
                    xs[:], x_sh[tc_ * 128:(tc_ + 1) * 128,
                                hb * 1024:(hb + 1) * 1024])
                x16t = p_x16.tile([128, 1024], f16, tag="x16",
                                  name=f"x16_{tc_}_{hb}")
                nc.scalar.copy(x16t[:], xs[:])
                th, tl = tc_ // 4, tc_ % 4
                dst = xTt[(th, pr)][:].rearrange("p (j c) -> p j c", j=16)
                nc.sync.dma_start_transpose(
                    dst[:, h * 8:h * 8 + 8, tl * 128:(tl + 1) * 128],
                    x16t[:])

            # B chunks feed wbT quarter tiles (JIT for GEMM2).
            wbT_tiles = {}
            b_staged = {}

            def b_compute(ic):
                q, oc = ic // 8, ic % 8
                if oc == 0:
                    wbT_tiles[q] = p_big.tile([128, 8192], f16, tag="big",
                                              name=f"wbT_{q}")
                wbs = soft24_chunk(b_staged.pop(ic), scale_b, p_wb,
                                   f"b{ic}")
                dst = wbT_tiles[q][:].rearrange("p (j c) -> p j c", j=8)
                nc.sync.dma_start_transpose(
                    dst[:, :, oc * 128:(oc + 1) * 128], wbs[:])

            # Issue order: x tiles for th0 first (GEMM1 lead-in), A chunks
            # paced with them; B loads staged early, B compute split around
            # the GEMM1 sweeps so each engine queue reaches the xp copies
            # exactly when its sweep ends.
            wa_sp = [None] * K_IN

            def a_chunk(k):
                wa_sp[k] = soft24_chunk(w_load(wa_d, k * 128, f"a{k}"),
                                        scale_a, p_wa, f"a{k}")

            for th in range(2):
                for pr in range(2):
                    xTt[(th, pr)] = p_big.tile([128, 8192], f16, tag="big",
                                               name=f"xT_{th}_{pr}")
            # lead-in: first x tiles for (th0, in-cols 0-1023) then interleave
            x_order = ([(tc_, hb) for hb in range(4) for tc_ in range(4)] +
                       [(tc_, hb) for hb in range(4) for tc_ in range(4, 8)])
            sched = []
            ai, xi = 0, 0
            for _ in range(4):
                sched.append(("x", x_order[xi])); xi += 1
            while ai < K_IN or xi < len(x_order):
                if ai < K_IN:
                    sched.append(("a", ai)); ai += 1
                if xi < len(x_order):
                    sched.append(("x", x_order[xi])); xi += 1
            for kind, arg in sched:
                if kind == "a":
                    a_chunk(arg)
                else:
                    x_tile(*arg)
            # stage all B loads early on the sync queue (they throttle on
            # the staging pool, never blocking anything urgent behind them)
            for ic in range(K_IN):
                b_staged[ic] = w_load(wb_d, ic * 128, f"b{ic}")

            # ---- GEMM1: x_projT[rank, tok] = A_sp^T @ x^T, two th sweeps ----
            xproj = {}

            def gemm1_sweep(th):
                accs = [p_ps.tile([128, 512], f32, tag="ps",
                                  name=f"g1_{th}_{rk}") for rk in range(K_RK)]
                for ic in range(K_IN):
                    pr, j = ic // 16, ic % 16
                    mv = xTt[(th, pr)][:, j * 512:(j + 1) * 512]
                    for rk in range(K_RK):
                        nc.tensor.matmul(
                            accs[rk][:],
                            wa_sp[ic][:, rk * 128:(rk + 1) * 128],
                            mv,
                            start=(ic == 0), stop=(ic == K_IN - 1))
                for rk in range(K_RK):
                    xp = p_xp.tile([128, 512], f16, tag="xp",
                                   name=f"xp_{th}_{rk}")
                    nc.vector.tensor_copy(out=xp[:], in_=accs[rk][:])
                    xproj[(th, rk)] = xp

            gemm1_sweep(0)
            for ic in range(0, 16):
                b_compute(ic)
            gemm1_sweep(1)
            for ic in range(16, K_IN):
                b_compute(ic)

            # ---- GEMM2 by quarter: out[tok, 1024-out-cols] ----
            for q in range(4):
                wbTq = wbT_tiles[q]
                for mt in range(8):
                    th = mt // 4
                    ml = mt % 4
                    ot = p_out.tile([128, 1024], f32, tag="oev",
                                    name=f"ot_{q}_{mt}")
                    for nbl in range(2):
                        acc2 = p_ps.tile([128, 512], f32, tag="ps",
                                         name=f"g2_{q}_{mt}_{nbl}")
                        for rk in range(K_RK):
                            nc.tensor.matmul(
                                acc2[:],
                                xproj[(th, rk)][:, ml * 128:(ml + 1) * 128],
                                wbTq[:, rk * 1024 + nbl * 512:
                                     rk * 1024 + nbl * 512 + 512],
                                start=(rk == 0), stop=(rk == K_RK - 1))
                        if bias_zero:
                            # last quarter: DVE is free by then, ACT is not
                            if q == 3:
                                nc.vector.tensor_copy(
                                    out=ot[:, nbl * 512:(nbl + 1) * 512],
                                    in_=acc2[:])
                            else:
                                nc.scalar.copy(
                                    ot[:, nbl * 512:(nbl + 1) * 512],
                                    acc2[:])
                        else:
                            nc.vector.tensor_tensor(
                                out=ot[:, nbl * 512:(nbl + 1) * 512],
                                in0=acc2[:],
                                in1=bias_bc[:, q * 1024 + nbl * 512:
                                            q * 1024 + nbl * 512 + 512],
                                op=Alu.add)
                    nc.scalar.dma_start(
                        out_d[mt * 128:(mt + 1) * 128,
                              q * 1024:(q + 1) * 1024],
                        ot[:])

    nc.compile()
    return nc


def kernel(x, weight_A, weight_B, bias, scale_A, scale_B):
    from concourse.bass_utils import run_bass_kernel_spmd

    x = np.ascontiguousarray(np.asarray(x, dtype=np.float32))
    weight_A = np.ascontiguousarray(np.asarray(weight_A, dtype=np.float32))
    weight_B = np.ascontiguousarray(np.asarray(weight_B, dtype=np.float32))
    bias = np.ascontiguousarray(np.asarray(bias, dtype=np.float32))
    sa = float(np.asarray(scale_A))
    sb = float(np.asarray(scale_B))
    bias_zero = bool(np.all(bias == 0.0))

    lead = x.shape[:-1]
    xf = x.reshape(-1, IN_F)
    assert xf.shape == (T_FULL, IN_F)

    key = (sa, sb, bias_zero)
    if key not in _BUILD_CACHE:
        _BUILD_CACHE[key] = _build(sa, sb, bias_zero)
    nc = _BUILD_CACHE[key]

    bias_row = bias.reshape(1, OUT_F)
    in_maps = []
    for c in range(N_CORES):
        in_maps.append({
            "x_sh": xf[c * TPC:(c + 1) * TPC],
            "wa_d": weight_A,
            "wb_d": weight_B,
            "bias_d": bias_row,
        })

    trace = os.environ.get("BASS_KERNEL_TRACE", "0") == "1"
    kwargs = {}
    if trace:
        _install_ntff_hook()
        kwargs["trace"] = True
        tmpdir = os.environ.get("BASS_KERNEL_TRACE_DIR")
        if tmpdir:
            os.makedirs(tmpdir, exist_ok=True)
            kwargs["tmpdir"] = tmpdir

    res = run_bass_kernel_spmd(nc, in_maps, core_ids=list(range(N_CORES)),
                               **kwargs)
    if trace:
        kernel.last_exec_time_ns = res.exec_time_ns

    out = np.empty((T_FULL, OUT_F), dtype=np.float32)
    for c in range(N_CORES):
        out[c * TPC:(c + 1) * TPC] = res.results[c]["out_d"]
    return out.reshape(*lead, OUT_F)


def _install_ntff_hook():
    """Provide antenv.axon_hooks (missing in this image) so trace=True works."""
    import types
    if "antenv.axon_hooks" in sys.modules:
        return
    try:
        from trn_agent_boot.trn_boot import _ntff_profile_via_ctypes
        hook = _ntff_profile_via_ctypes("/opt/axon/libaxon_pjrt.so")
    except Exception:
        hook = None
    mod = types.ModuleType("antenv.axon_hooks")
    mod.get_axon_ntff_profile_hook = lambda: hook
    mod.set_axon_ntff_profile_hook = lambda h: None
    import antenv  # noqa: F401
    sys.modules["antenv.axon_hooks"] = mod
